# revision 1
# baseline (speedup 1.0000x reference)
"""Distributed Bass kernel for nn_AdaGNN (2-layer GAT + MLP heads + CE losses).

Strategy (8 NeuronCores, SPMD):
  - Nodes assigned to 8 cores x NT tiles of 128 by a load-balancing packer
    (equal edge counts per tile). Output is permutation invariant.
  - Per layer: dense per-node transform producing a 128-wide (256B) gather-table
    row [feat(64) | a_src(8) | a_dst(8) | pad] bf16 -> AllGather -> per-edge
    dma_gather (int16 indices; 4 source windows of TBL/4 rows each; dst rows
    from the local table) -> segment softmax via exp (value ranges are small;
    max-subtraction unnecessary) -> weighted segment-sum via one-hot matmuls
    on TensorE -> normalize.
  - Edge chunks of 128 are keyed (tile, window, q) with a per-tile/window
    chunk schedule shared by all cores (SPMD-static); tiles are grouped into
    batches of identical schedule vectors so gather outputs stay contiguous.
  - Layer 2 aggregates per-head-weighted 64-dim inputs (512-wide messages) and
    applies the reshuffled W2 (mean over heads folded in) after aggregation.
  - MLP heads + masked CE per dst tile; partial sums AllReduced; final scalar
    computed on device.
"""

import math
import numpy as np
import ml_dtypes

import concourse.bass as bass
import concourse.tile as tile
from concourse import mybir
from concourse.bacc import Bacc
from concourse.bass_utils import run_bass_kernel_spmd

BF16 = mybir.dt.bfloat16
F32 = mybir.dt.float32
I16 = mybir.dt.int16
P = 128
NCORES = 8
NW = 4          # gather windows
RW = 128        # table row width (elements, bf16) = 256B
AF = mybir.ActivationFunctionType
OP = mybir.AluOpType

nbf = ml_dtypes.bfloat16


# ----------------------------------------------------------------------------
# Host-side graph preprocessing
# ----------------------------------------------------------------------------

def _prep(inputs, tiles_per_batch=5):
    x = np.asarray(inputs["x"], np.float32)
    ei = np.asarray(inputs["edge_index"], np.int32)
    N, D_IN = x.shape
    NPC = N // NCORES
    NT = math.ceil(NPC / P)
    NPAD = NT * P
    TBL = NCORES * NPAD
    WIN = TBL // NW
    NBINS = NCORES * NT

    src = np.concatenate([ei[0], np.arange(N, dtype=np.int32)])
    dst = np.concatenate([ei[1], np.arange(N, dtype=np.int32)])

    # ---- balanced node -> (core, tile, slot) assignment ----
    import heapq
    deg = np.bincount(dst, minlength=N).astype(np.int64)
    order_n = np.argsort(-deg, kind="stable")
    heap = [(0, b) for b in range(NBINS)]
    heapq.heapify(heap)
    bin_cnt = np.zeros(NBINS, np.int64)
    bin_edges = np.zeros(NBINS, np.int64)
    node_bin = np.zeros(N, np.int32)
    node_slot = np.zeros(N, np.int32)
    for n in order_n:
        while True:
            e, b = heapq.heappop(heap)
            if e == bin_edges[b] and bin_cnt[b] < P:
                break
        node_bin[n] = b
        node_slot[n] = bin_cnt[b]
        bin_cnt[b] += 1
        bin_edges[b] += deg[n]
        if bin_cnt[b] < P:
            heapq.heappush(heap, (int(bin_edges[b]), b))
    node_core = node_bin // NT
    node_tile = node_bin % NT

    rowpos = node_core.astype(np.int64) * NPAD + node_tile * P + node_slot

    core_of = node_core[dst]
    tile_of = node_tile[dst]
    loc_of = node_slot[dst]
    srow = rowpos[src]
    win_of = (srow // WIN).astype(np.int32)

    # per (core, tile, window) counts -> shared schedule
    cnt = np.zeros((NCORES, NT, NW), np.int64)
    np.add.at(cnt, (core_of, tile_of, win_of), 1)
    chs = np.ceil(cnt / P).astype(np.int64).max(axis=0)  # [NT, NW]
    chs[:, 0] = np.maximum(1, chs[:, 0])  # every tile aggregates >= 1 chunk

    # group tiles by schedule vector; build batches of identical structure
    keys = [tuple(chs[t]) for t in range(NT)]
    order_t = sorted(range(NT), key=lambda t: (keys[t], t))
    batches = []  # (tiles, cvec)
    i = 0
    while i < NT:
        j = i
        while (j < NT and keys[order_t[j]] == keys[order_t[i]]
               and j - i < tiles_per_batch):
            j += 1
        batches.append(([order_t[k] for k in range(i, j)],
                        np.array(keys[order_t[i]], np.int64)))
        i = j

    # chunk bookkeeping in batch order
    CH = 0
    CHW = np.zeros(NW, np.int64)
    binfo = []  # (c0, cw0[4], tiles, cvec)
    for tiles, cvec in batches:
        binfo.append((CH, CHW.copy(), tiles, cvec))
        CH += int(cvec.sum()) * len(tiles)
        CHW += cvec * len(tiles)
    CH = int(CH)

    # per-core edge arrays
    per_core = []
    for c in range(NCORES):
        sel = core_of == c
        s_row, t_c, l_c, w_c = srow[sel], tile_of[sel], loc_of[sel], win_of[sel]
        srcw = [np.zeros(max(1, int(CHW[w])) * P, np.int16) for w in range(NW)]
        dsti = np.zeros(CH * P, np.int16)
        dstloc = np.full((CH, P), -1.0, np.float32)
        for (c0, cw0, tiles, cvec) in binfo:
            S = int(cvec.sum())
            for i_t, t in enumerate(tiles):
                off_w = 0
                for w in range(NW):
                    cw = int(cvec[w])
                    if cw == 0:
                        continue
                    m = (t_c == t) & (w_c == w)
                    k = int(m.sum())
                    assert k <= cw * P, (k, cw)
                    rows_l = (s_row[m] - w * WIN).astype(np.int16)
                    lt = l_c[m]
                    gp = c0 + i_t * S + off_w          # global chunk position
                    wp = int(cw0[w]) + i_t * cw        # window-local position
                    j = np.arange(k)
                    srcw[w][(wp + j // P) * P + (j % P)] = rows_l
                    dsti[(gp + j // P) * P + (j % P)] = (t * P + lt).astype(np.int16)
                    dstloc[gp + j // P, j % P] = lt
                    off_w += cw

        def wrap(ids):
            a = ids.reshape(-1, 16).T.copy()
            return np.tile(a, (8, 1)).astype(np.int16)

        per_core.append(([wrap(srcw[w]) for w in range(NW)],
                         wrap(dsti), dstloc.T.copy()))

    # ----- weights / constants (replicated) -----
    f32 = np.float32
    W1 = np.asarray(inputs["W1"], f32)
    as1 = np.asarray(inputs["att_src1"], f32)
    ad1 = np.asarray(inputs["att_dst1"], f32)
    W1h = W1.reshape(D_IN, 8, 8)
    wtab1 = np.concatenate(
        [W1, np.einsum("khc,hc->kh", W1h, as1), np.einsum("khc,hc->kh", W1h, ad1)], 1
    )  # [D_IN, 80]
    KA = 128 if D_IN > 128 else D_IN
    KB = D_IN - KA

    W2 = np.asarray(inputs["W2"], f32)
    as2 = np.asarray(inputs["att_src2"], f32)
    ad2 = np.asarray(inputs["att_dst2"], f32)
    W2h = W2.reshape(64, 8, 64)
    wsd2 = np.concatenate(
        [np.einsum("khc,hc->kh", W2h, as2), np.einsum("khc,hc->kh", W2h, ad2)], 1
    )  # [64, 16]
    wbig = (W2h.transpose(1, 0, 2).reshape(512, 64) / 8.0)
    wbig_dev = wbig.reshape(4, 128, 64).transpose(1, 0, 2).reshape(128, 256)

    consts = {
        "wtab1": wtab1.astype(nbf),
        "wsd2": wsd2.astype(nbf),
        "wbig": wbig_dev.astype(nbf),
        "tsw1": np.asarray(inputs["ts_w1"], f32).astype(nbf),
        "clsw1": np.asarray(inputs["cls_w1"], f32).astype(nbf),
        "tsw2": np.asarray(inputs["ts_w2"], f32).astype(nbf),
        "clsw2": np.asarray(inputs["cls_w2"], f32).astype(nbf),
        "b1r": np.tile(np.asarray(inputs["b1"], f32)[None, :], (P, 1)),
        "b2r": np.tile(np.asarray(inputs["b2"], f32)[None, :], (P, 1)),
        "tsb1c": np.asarray(inputs["ts_b1"], f32).reshape(64, 1),
        "clsb1c": np.asarray(inputs["cls_b1"], f32).reshape(64, 1),
        "tsb2c": np.pad(np.asarray(inputs["ts_b2"], f32), (0, 3)).reshape(8, 1),
        "clsb2c": np.pad(np.asarray(inputs["cls_b2"], f32), (0, 6)).reshape(8, 1),
        "iota": np.tile(np.arange(P, dtype=f32)[None, :], (P, 1)).astype(nbf),
        "ident": np.eye(P, dtype=f32).astype(nbf),
        "identf": np.eye(P, dtype=f32),
        "ones": np.ones((P, 1), f32),
    }

    tst = np.asarray(inputs["timestamp_target"], np.int64)
    clt = np.asarray(inputs["node_target"], np.int64)
    msk = np.asarray(inputs["node_mask"]).astype(f32)

    in_maps = []
    pos_in_core = node_tile.astype(np.int64) * P + node_slot
    for c in range(NCORES):
        srcw, dsti, dstloc = per_core[c]
        mine = np.nonzero(node_core == c)[0]
        pos = pos_in_core[mine]
        xT = np.zeros((D_IN, NPAD), f32)
        xT[:, pos] = x[mine].T
        valid = np.zeros(NPAD, bool)
        valid[pos] = True
        g_ts = np.zeros(NPAD, np.int64)
        g_ts[pos] = tst[mine]
        g_cl = np.zeros(NPAD, np.int64)
        g_cl[pos] = clt[mine]
        g_mk = np.zeros(NPAD, f32)
        g_mk[pos] = msk[mine]
        rows = np.arange(NPAD)
        ohts = np.zeros((NPAD, 5), f32)
        ohts[rows, g_ts] = 1.0
        ohcl = np.zeros((NPAD, 2), f32)
        ohcl[rows, g_cl] = 1.0

        def pm(a, w):
            return a.reshape(NT, P, w).transpose(1, 0, 2).reshape(P, NT * w).copy()

        m = {
            "xT": xT.astype(nbf),
            "dsti": dsti,
            "dstloc": dstloc.astype(nbf),
            "ohts": pm(ohts, 5),
            "ohcl": pm(ohcl, 2),
            "vm": pm(np.stack([valid.astype(f32), (g_mk * valid).astype(f32)],
                              axis=1), 2),
        }
        for w in range(NW):
            m[f"srcw{w}"] = srcw[w]
        m.update(consts)
        in_maps.append(m)

    cfg = dict(N=N, D_IN=D_IN, NPC=NPC, NT=NT, NPAD=NPAD, TBL=TBL, WIN=WIN,
               CH=CH, CHW=CHW, KA=KA, KB=KB, binfo=binfo)
    return cfg, in_maps


# ----------------------------------------------------------------------------
# Device graph
# ----------------------------------------------------------------------------

def _build(cfg):
    import os
    STOPAT = int(os.environ.get("STOPAT", "99"))
    N, D_IN = cfg["N"], cfg["D_IN"]
    NT, NPAD, TBL, WIN = cfg["NT"], cfg["NPAD"], cfg["TBL"], cfg["WIN"]
    CH, CHW = cfg["CH"], cfg["CHW"]
    KA, KB = cfg["KA"], cfg["KB"]
    binfo = cfg["binfo"]
    RG = [list(range(NCORES))]

    kbmax = max(int(cv.sum()) * len(tl) for (_, _, tl, cv) in binfo)

    nc = Bacc("TRN2", target_bir_lowering=False, num_devices=NCORES)

    ein = lambda name, shp, dt: nc.dram_tensor(name, shp, dt, kind="ExternalInput")
    xT_d = ein("xT", [D_IN, NPAD], BF16)
    srcw_d = [ein(f"srcw{w}", [P, max(1, int(CHW[w])) * 8], I16) for w in range(NW)]
    dsti_d = ein("dsti", [P, CH * 8], I16)
    dstloc_d = ein("dstloc", [P, CH], BF16)
    ohts_d = ein("ohts", [P, NT * 5], F32)
    ohcl_d = ein("ohcl", [P, NT * 2], F32)
    vm_d = ein("vm", [P, NT * 2], F32)
    wtab1_d = ein("wtab1", [D_IN, 80], BF16)
    wsd2_d = ein("wsd2", [64, 16], BF16)
    wbig_d = ein("wbig", [P, 256], BF16)
    tsw1_d = ein("tsw1", [64, 64], BF16)
    clsw1_d = ein("clsw1", [64, 64], BF16)
    tsw2_d = ein("tsw2", [64, 5], BF16)
    clsw2_d = ein("clsw2", [64, 2], BF16)
    b1r_d = ein("b1r", [P, 64], F32)
    b2r_d = ein("b2r", [P, 64], F32)
    tsb1c_d = ein("tsb1c", [64, 1], F32)
    clsb1c_d = ein("clsb1c", [64, 1], F32)
    tsb2c_d = ein("tsb2c", [8, 1], F32)
    clsb2c_d = ein("clsb2c", [8, 1], F32)
    iota_d = ein("iota", [P, P], BF16)
    identf_d = ein("identf", [P, P], F32)
    ident_d = ein("ident", [P, P], BF16)
    ones_d = ein("ones", [P, 1], F32)

    out_d = nc.dram_tensor("out", [1, 1], F32, kind="ExternalOutput")

    tbl1_loc = nc.dram_tensor("tbl1_loc", [NPAD, RW], BF16)
    tbl1_full = nc.dram_tensor("tbl1_full", [TBL, RW], BF16, addr_space="Shared")
    tbl2_loc = nc.dram_tensor("tbl2_loc", [NPAD, RW], BF16)
    tbl2_full = nc.dram_tensor("tbl2_full", [TBL, RW], BF16, addr_space="Shared")
    ar_in = nc.dram_tensor("ar_in", [1, 8], F32)
    ar_out = nc.dram_tensor("ar_out", [1, 8], F32, addr_space="Shared")

    with tile.TileContext(nc) as tc:
        with (
            tc.tile_pool(name="const", bufs=1) as cp,
            tc.tile_pool(name="sbuf", bufs=2) as sp,
            tc.tile_pool(name="stage", bufs=2) as stp,
            tc.tile_pool(name="psum", bufs=2, space="PSUM") as pp,
        ):
            # ---------------- constants to SBUF ----------------
            def ld(t, dram, shape, dt=BF16):
                s = cp.tile(shape, dt, tag=t, name=t)
                nc.sync.dma_start(out=s[: shape[0]], in_=dram[:])
                return s

            wt1a = cp.tile([KA, 80], BF16, tag="wt1a")
            nc.sync.dma_start(out=wt1a[:], in_=wtab1_d[0:KA, :])
            if KB:
                wt1b = cp.tile([max(KB, 32), 80], BF16, tag="wt1b")
                nc.sync.dma_start(out=wt1b[:KB], in_=wtab1_d[KA:D_IN, :])
            wsd2 = ld("wsd2", wsd2_d, [64, 16])
            wbig = ld("wbig", wbig_d, [P, 256])
            tsw1 = ld("tsw1", tsw1_d, [64, 64])
            clsw1 = ld("clsw1", clsw1_d, [64, 64])
            tsw2 = ld("tsw2", tsw2_d, [64, 5])
            clsw2 = ld("clsw2", clsw2_d, [64, 2])
            b1r = ld("b1r", b1r_d, [P, 64], F32)
            b2r = ld("b2r", b2r_d, [P, 64], F32)
            tsb1c = ld("tsb1c", tsb1c_d, [64, 1], F32)
            clsb1c = ld("clsb1c", clsb1c_d, [64, 1], F32)
            tsb2c = ld("tsb2c", tsb2c_d, [8, 1], F32)
            clsb2c = ld("clsb2c", clsb2c_d, [8, 1], F32)
            iota = ld("iota", iota_d, [P, P])
            ident = ld("ident", ident_d, [P, P])
            identf = ld("identf", identf_d, [P, P], F32)
            ones = ld("ones", ones_d, [P, 1], F32)
            srcw = [ld(f"srcw{w}", srcw_d[w], [P, max(1, int(CHW[w])) * 8], I16)
                    for w in range(NW)]
            dsti = ld("dsti", dsti_d, [P, CH * 8], I16)
            dstloc = ld("dstloc", dstloc_d, [P, CH])
            ohts = ld("ohts", ohts_d, [P, NT * 5], F32)
            ohcl = ld("ohcl", ohcl_d, [P, NT * 2], F32)
            vm = ld("vm", vm_d, [P, NT * 2], F32)

            acc = cp.tile([P, 4], F32, tag="acc")
            nc.vector.memset(acc[:], 0.0)

            # ---------------- phase A: layer-1 table ----------------
            WG = 7  # tiles per table-write group
            for g0 in range(0, NT, WG):
                gn = min(WG, NT - g0)
                tbl_sb = stp.tile([P, WG * RW], BF16, tag="tbl_w", name="tblw")
                nc.vector.memset(tbl_sb[:], 0.0)
                for ti in range(gn):
                    t = g0 + ti
                    xa = sp.tile([P, P], BF16, tag="xa")
                    nc.sync.dma_start(out=xa[:], in_=xT_d[0:KA, t * P:(t + 1) * P])
                    pA = pp.tile([P, 512], F32, tag="agg", bufs=2)
                    if KB:
                        xb = sp.tile([max(KB, 32), P], BF16, tag="xb")
                        nc.sync.dma_start(out=xb[:KB], in_=xT_d[KA:D_IN, t * P:(t + 1) * P])
                        nc.tensor.matmul(pA[:, 0:80], lhsT=xa[:], rhs=wt1a[:], start=True, stop=False)
                        nc.tensor.matmul(pA[:, 0:80], lhsT=xb[:KB], rhs=wt1b[:KB], start=False, stop=True)
                    else:
                        nc.tensor.matmul(pA[:, 0:80], lhsT=xa[:], rhs=wt1a[:], start=True, stop=True)
                    nc.scalar.activation(tbl_sb[:, ti * RW:ti * RW + 80], pA[:, 0:80], AF.Copy)
                tdst = tbl1_loc[:].rearrange("(t p) w -> p t w", p=P)[:, g0:g0 + gn, :]
                nc.sync.dma_start(out=tdst, in_=tbl_sb[:].rearrange("p (t w) -> p t w", w=RW)[:, 0:gn, :])

            if STOPAT >= 1:
                nc.gpsimd.collective_compute(
                    "AllGather", OP.bypass, ins=[tbl1_loc[:]], outs=[tbl1_full[:]],
                    replica_groups=RG,
                )

            # ---------------- edge phases ----------------
            def edge_layer(layer, tbl_full, tbl_loc, finalize):
                WM = 72 if layer == 1 else 520
                FW = 64 if layer == 1 else 512
                for (c0, cw0, tiles, cvec) in binfo:
                    nb = len(tiles)
                    S = int(cvec.sum())
                    kb = nb * S
                    # window-major run offsets (in chunks) inside batch slabs
                    woff = []
                    o = 0
                    for w in range(NW):
                        woff.append(o)
                        o += nb * int(cvec[w])
                    gm = sp.tile([P, kbmax * RW], BF16, tag="gm")
                    for w in range(NW):
                        cw = int(cvec[w])
                        if cw == 0:
                            continue
                        kbw = nb * cw
                        nc.gpsimd.dma_gather(
                            out_ap=gm[:, woff[w] * RW:(woff[w] + kbw) * RW]
                                .rearrange("p (c e) -> p c e", e=RW),
                            in_ap=tbl_full[w * WIN:(w + 1) * WIN, :],
                            idxs_ap=srcw[w][:, int(cw0[w]) * 8:(int(cw0[w]) + kbw) * 8],
                            num_idxs=kbw * P, num_idxs_reg=kbw * P, elem_size=RW,
                            single_packet=False)
                    gad = sp.tile([P, kbmax * RW], BF16, tag="gad")
                    nc.gpsimd.dma_gather(
                        out_ap=gad[:, 0:kb * RW].rearrange("p (c e) -> p c e", e=RW),
                        in_ap=tbl_loc[:],
                        idxs_ap=dsti[:, c0 * 8:(c0 + kb) * 8],
                        num_idxs=kb * P, num_idxs_reg=kb * P, elem_size=RW,
                        single_packet=False)

                    alpha = sp.tile([P, kbmax * 8], F32, tag="alpha")
                    lrel = sp.tile([P, kbmax * 8], F32, tag="lrel")
                    ea = sp.tile([P, kbmax * 8], BF16, tag="ea")
                    msg = sp.tile([P, kbmax * WM], BF16, tag="msg")
                    oh = sp.tile([P, kbmax * P], BF16, tag="oh")
                    off_w = 0
                    for w in range(NW):
                        cw = int(cvec[w])
                        if cw == 0:
                            continue
                        kbw = nb * cw
                        wo = woff[w]
                        g4 = gm[:, wo * RW:(wo + kbw) * RW].rearrange(
                            "p (n s e) -> p n s e", s=cw, e=RW)
                        ad4 = gad[:, 0:kb * RW].rearrange(
                            "p (n s e) -> p n s e", s=S, e=RW)[:, :, off_w:off_w + cw, 72:80]
                        al4 = alpha[:, wo * 8:(wo + kbw) * 8].rearrange(
                            "p (n s e) -> p n s e", s=cw, e=8)
                        nc.vector.tensor_tensor(out=al4, in0=g4[:, :, :, 64:72],
                                                in1=ad4, op=OP.add)
                        nc.vector.scalar_tensor_tensor(
                            out=lrel[:, wo * 8:(wo + kbw) * 8],
                            in0=alpha[:, wo * 8:(wo + kbw) * 8], scalar=0.2,
                            in1=alpha[:, wo * 8:(wo + kbw) * 8], op0=OP.mult, op1=OP.max)
                        nc.scalar.activation(ea[:, wo * 8:(wo + kbw) * 8],
                                             lrel[:, wo * 8:(wo + kbw) * 8], AF.Exp)
                        ea3 = ea[:, wo * 8:(wo + kbw) * 8].rearrange("p (k e) -> p k e", e=8)
                        ms3 = msg[:, wo * WM:(wo + kbw) * WM].rearrange("p (k e) -> p k e", e=WM)
                        gk = gm[:, wo * RW:(wo + kbw) * RW].rearrange("p (k e) -> p k e", e=RW)
                        if layer == 1:
                            nc.vector.tensor_tensor(
                                out=ms3[:, :, 0:64].rearrange("p k (h c) -> p k h c", h=8),
                                in0=gk[:, :, 0:64].rearrange("p k (h c) -> p k h c", h=8),
                                in1=ea3.unsqueeze(3).to_broadcast([P, kbw, 8, 8]),
                                op=OP.mult,
                            )
                        else:
                            nc.vector.tensor_tensor(
                                out=ms3[:, :, 0:512].rearrange("p k (h c) -> p k h c", h=8),
                                in0=gk[:, :, 0:64].unsqueeze(2).to_broadcast([P, kbw, 8, 64]),
                                in1=ea3.unsqueeze(3).to_broadcast([P, kbw, 8, 64]),
                                op=OP.mult,
                            )
                        nc.vector.tensor_copy(ms3[:, :, WM - 8:WM], ea3)
                        dl3 = dstloc[:, c0:c0 + kb].rearrange("p (n s) -> p n s", s=S)[
                            :, :, off_w:off_w + cw]
                        nc.vector.tensor_tensor(
                            out=oh[:, wo * P:(wo + kbw) * P].rearrange(
                                "p (n s e) -> p n s e", s=cw, e=P),
                            in0=dl3.unsqueeze(3).to_broadcast([P, nb, cw, P]),
                            in1=iota[:].unsqueeze(1).unsqueeze(2).to_broadcast([P, nb, cw, P]),
                            op=OP.is_equal,
                        )
                        off_w += cw

                    for i_t, t in enumerate(tiles):
                        pz = pp.tile([P, 512], F32, tag="agg", bufs=2, name="pz")
                        pd = (pp.tile([P, 8], F32, tag="den", bufs=1, name="pd")
                              if layer == 2 else None)
                        first = True
                        done = 0
                        for w in range(NW):
                            cw = int(cvec[w])
                            for q in range(cw):
                                jj = woff[w] + i_t * cw + q
                                ohj = oh[:, jj * P:(jj + 1) * P]
                                mj = msg[:, jj * WM:(jj + 1) * WM]
                                done += 1
                                st, fi = first, (done == S)
                                nc.tensor.matmul(
                                    pz[:, 0:FW + (8 if layer == 1 else 0)],
                                    lhsT=ohj,
                                    rhs=mj[:, 0:FW + (8 if layer == 1 else 0)],
                                    start=st, stop=fi)
                                if layer == 2:
                                    nc.tensor.matmul(pd[:], lhsT=ohj,
                                                     rhs=mj[:, 512:520],
                                                     start=st, stop=fi)
                                first = False
                        finalize(t, pz, pd)

            # ---------------- layer-1 finalize: h1, layer-2 table ----------------
            tbl2_stage = {}
            t2_state = {"n": 0, "rows": []}

            def fin1(t, pz, pd):
                rin = sp.tile([P, 8], F32, tag="rin")
                nc.vector.tensor_scalar_add(rin[:], pz[:, 64:72], 1e-16)
                rcp = sp.tile([P, 8], F32, tag="rcp")
                nc.vector.reciprocal(rcp[:], rin[:])
                h1f = sp.tile([P, 64], F32, tag="h1f")
                nc.vector.tensor_tensor(
                    out=h1f[:].rearrange("p (h c) -> p h c", h=8),
                    in0=pz[:, 0:64].rearrange("p (h c) -> p h c", h=8),
                    in1=rcp[:].unsqueeze(2).to_broadcast([P, 8, 8]),
                    op=OP.mult,
                )
                ti = t2_state["n"] % 7
                if ti == 0:
                    tbl2_stage[0] = stp.tile([P, 7 * RW], BF16, tag="tbl2_w",
                                             name="tbl2w")
                    nc.vector.memset(tbl2_stage[0][:], 0.0)
                trow = tbl2_stage[0]
                nc.vector.tensor_add(trow[:, ti * RW:ti * RW + 64], h1f[:], b1r[:])
                tp = pp.tile([P, P], BF16, tag="tpb", bufs=2)
                nc.tensor.transpose(tp[0:64, :], trow[:, ti * RW:ti * RW + 64], ident[:])
                h1T = sp.tile([64, P], BF16, tag="h1T")
                nc.scalar.activation(h1T[:], tp[0:64, :], AF.Copy)
                pf = pp.tile([P, 16], F32, tag="hp", bufs=1)
                nc.tensor.matmul(pf[:], lhsT=h1T[:], rhs=wsd2[:], start=True, stop=True)
                nc.scalar.activation(trow[:, ti * RW + 64:ti * RW + 80], pf[:], AF.Copy)
                t2_state["rows"].append(t)
                t2_state["n"] += 1
                if ti == 6 or t2_state["n"] == NT:
                    gn = ti + 1
                    rows = t2_state["rows"][-gn:]
                    tdst = tbl2_loc[:].rearrange("(t p) w -> p t w", p=P)
                    tsrc = trow[:].rearrange("p (t w) -> p t w", w=RW)
                    for k, tt in enumerate(rows):
                        nc.sync.dma_start(out=tdst[:, tt:tt + 1, :],
                                          in_=tsrc[:, k:k + 1, :])

            if STOPAT >= 2:
                edge_layer(1, tbl1_full, tbl1_loc, fin1)

            if STOPAT >= 3:
                nc.gpsimd.collective_compute(
                    "AllGather", OP.bypass, ins=[tbl2_loc[:]], outs=[tbl2_full[:]],
                    replica_groups=RG,
                )

            # ---------------- layer-2 finalize: h2, MLPs, CE ----------------
            FIN2LVL = int(os.environ.get("FIN2LVL", "9"))

            def fin2(t, pz, pd):
                if FIN2LVL == 0:
                    dump = sp.tile([P, 8], F32, tag="rin")
                    nc.scalar.activation(dump[:], pd[:], AF.Copy)
                    dump2 = sp.tile([P, 8], F32, tag="rcp")
                    nc.scalar.activation(dump2[:], pz[:, 0:8], AF.Copy)
                    return
                rin = sp.tile([P, 8], F32, tag="rin")
                nc.vector.tensor_scalar_add(rin[:], pd[:], 1e-16)
                rcp = sp.tile([P, 8], F32, tag="rcp")
                nc.vector.reciprocal(rcp[:], rin[:])
                zn = sp.tile([P, 512], BF16, tag="zn")
                nc.vector.tensor_tensor(
                    out=zn[:].rearrange("p (h c) -> p h c", h=8),
                    in0=pz[:].rearrange("p (h c) -> p h c", h=8),
                    in1=rcp[:].unsqueeze(2).to_broadcast([P, 8, 64]),
                    op=OP.mult,
                )
                if FIN2LVL <= 1:
                    return
                hp = pp.tile([P, 64], F32, tag="hp", bufs=1)
                for k in range(4):
                    tpz = pp.tile([P, P], BF16, tag="tpb", bufs=2)
                    nc.tensor.transpose(tpz[:], zn[:, k * P:(k + 1) * P], ident[:])
                    zT = sp.tile([P, P], BF16, tag="zT")
                    nc.scalar.activation(zT[:], tpz[:], AF.Copy)
                    nc.tensor.matmul(hp[:], lhsT=zT[:], rhs=wbig[:, k * 64:(k + 1) * 64],
                                     start=(k == 0), stop=(k == 3))
                h2 = sp.tile([P, 64], BF16, tag="h2")
                nc.vector.tensor_add(h2[:], hp[:], b2r[:])
                if FIN2LVL <= 2:
                    return
                tph = pp.tile([P, P], BF16, tag="tpb", bufs=2)
                nc.tensor.transpose(tph[0:64, :], h2[:], ident[:])
                h2T = sp.tile([64, P], BF16, tag="h2T")
                nc.scalar.activation(h2T[:], tph[0:64, :], AF.Copy)

                pa1 = pp.tile([64, P], F32, tag="tp", bufs=2)
                nc.tensor.matmul(pa1[:], lhsT=tsw1[:], rhs=h2T[:], start=True, stop=True)
                a1T = sp.tile([64, P], BF16, tag="a1T")
                nc.scalar.activation(a1T[:], pa1[:], AF.Relu, bias=tsb1c[:, 0:1])
                pc1 = pp.tile([64, P], F32, tag="tp", bufs=2)
                nc.tensor.matmul(pc1[:], lhsT=clsw1[:], rhs=h2T[:], start=True, stop=True)
                c1T = sp.tile([64, P], BF16, tag="c1T")
                nc.scalar.activation(c1T[:], pc1[:], AF.Relu, bias=clsb1c[:, 0:1])

                if FIN2LVL <= 3:
                    return
                lgt = pp.tile([8, P], F32, tag="tp", bufs=2)
                nc.tensor.matmul(lgt[0:5, :], lhsT=tsw2[:], rhs=a1T[:], start=True, stop=True)
                lgc = pp.tile([8, P], F32, tag="tp", bufs=2)
                nc.tensor.matmul(lgc[0:2, :], lhsT=clsw2[:], rhs=c1T[:], start=True, stop=True)
                lgs = sp.tile([64, P], F32, tag="lgs")
                nc.vector.memset(lgs[:], 0.0)
                nc.scalar.activation(lgs[0:5, :], lgt[0:5, :], AF.Identity, bias=tsb2c[0:5, 0:1])
                nc.scalar.activation(lgs[32:34, :], lgc[0:2, :], AF.Identity, bias=clsb2c[0:2, 0:1])
                tlg = pp.tile([P, 64], F32, tag="tp", bufs=2)
                nc.tensor.matmul(tlg[:], lhsT=lgs[:], rhs=identf[0:64, 0:64],
                                 is_transpose=True, start=True, stop=True)
                tlg2 = tlg[:, 32:40]

                if FIN2LVL <= 4:
                    return
                s2 = sp.tile([P, 2], F32, tag="s2")
                ex = sp.tile([P, 8], F32, tag="ex")
                nc.scalar.activation(ex[:, 0:5], tlg[:, 0:5], AF.Exp, accum_out=s2[:, 0:1])
                ex2 = sp.tile([P, 8], F32, tag="ex2")
                nc.scalar.activation(ex2[:, 0:2], tlg2[:, 0:2], AF.Exp, accum_out=s2[:, 1:2])
                lse = sp.tile([P, 2], F32, tag="lse")
                nc.scalar.activation(lse[:], s2[:], AF.Ln)
                pk = sp.tile([P, 8], F32, tag="pk")
                pks = sp.tile([P, 2], F32, tag="pks")
                nc.vector.tensor_tensor(out=pk[:, 0:5], in0=tlg[:, 0:5],
                                        in1=ohts[:, t * 5:(t + 1) * 5], op=OP.mult)
                nc.vector.tensor_tensor(out=pk[:, 5:7], in0=tlg2[:, 0:2],
                                        in1=ohcl[:, t * 2:(t + 1) * 2], op=OP.mult)
                nc.vector.reduce_sum(pks[:, 0:1], pk[:, 0:5], axis=mybir.AxisListType.X)
                nc.vector.reduce_sum(pks[:, 1:2], pk[:, 5:7], axis=mybir.AxisListType.X)
                ce = sp.tile([P, 2], F32, tag="ce")
                nc.vector.tensor_sub(ce[:], lse[:], pks[:])
                cem = sp.tile([P, 2], F32, tag="cem")
                nc.vector.tensor_tensor(out=cem[:], in0=ce[:], in1=vm[:, t * 2:t * 2 + 2],
                                        op=OP.mult)
                nc.vector.tensor_add(acc[:, 0:2], acc[:, 0:2], cem[:])
                nc.vector.tensor_add(acc[:, 2:3], acc[:, 2:3], vm[:, t * 2 + 1:t * 2 + 2])

            if STOPAT >= 4:
                edge_layer(2, tbl2_full, tbl2_loc, fin2)

            # ---------------- final reduction ----------------
            pfin = pp.tile([1, 8], F32, tag="tp", bufs=2)
            nc.tensor.matmul(pfin[0:1, 0:3], lhsT=ones[:], rhs=acc[:, 0:3],
                             start=True, stop=True)
            fin_sb = cp.tile([1, 8], F32, tag="fin")
            nc.vector.memset(fin_sb[:], 0.0)
            nc.scalar.activation(fin_sb[0:1, 0:3], pfin[0:1, 0:3], AF.Copy)
            nc.sync.dma_start(out=ar_in[:], in_=fin_sb[:])
            nc.gpsimd.collective_compute(
                "AllReduce", OP.add, ins=[ar_in[:]], outs=[ar_out[:]],
                replica_groups=RG,
            )
            tot = cp.tile([1, 8], F32, tag="tot")
            nc.sync.dma_start(out=tot[:], in_=ar_out[:])
            rcpm = cp.tile([1, 1], F32, tag="rcpm")
            nc.vector.reciprocal(rcpm[:], tot[:, 2:3])
            lcl = cp.tile([1, 1], F32, tag="lcl")
            nc.vector.tensor_tensor(out=lcl[:], in0=tot[:, 1:2], in1=rcpm[:], op=OP.mult)
            lts = cp.tile([1, 1], F32, tag="lts")
            nc.vector.tensor_scalar_mul(lts[:], tot[:, 0:1], 1.0 / N)
            res = cp.tile([1, 1], F32, tag="res")
            nc.vector.tensor_add(res[:], lcl[:], lts[:])
            nc.sync.dma_start(out=out_d[:], in_=res[:])

    nc.compile()
    return nc


# ----------------------------------------------------------------------------
# Entry points
# ----------------------------------------------------------------------------

def _run(inputs, trace=False):
    cfg, in_maps = _prep(inputs)
    nc = _build(cfg)
    try:
        r = run_bass_kernel_spmd(nc, in_maps, core_ids=list(range(NCORES)), trace=trace)
    except ModuleNotFoundError:
        r = run_bass_kernel_spmd(nc, in_maps, core_ids=list(range(NCORES)), trace=False)
    out = np.asarray(r.results[0]["out"], np.float32).reshape(())
    return out, r


def kernel(**inputs):
    out, _ = _run(inputs, trace=False)
    return out


def _build_null(cfg):
    """Same I/O signature, trivial compute — for dispatch/transfer baseline."""
    N, D_IN = cfg["N"], cfg["D_IN"]
    NT, NPAD, TBL, WIN = cfg["NT"], cfg["NPAD"], cfg["TBL"], cfg["WIN"]
    CH, CHW = cfg["CH"], cfg["CHW"]
    binfo = cfg["binfo"]
    nc = Bacc("TRN2", target_bir_lowering=False, num_devices=NCORES)
    ein = lambda name, shp, dt: nc.dram_tensor(name, shp, dt, kind="ExternalInput")
    xT_d = ein("xT", [D_IN, NPAD], BF16)
    for w in range(NW):
        ein(f"srcw{w}", [P, max(1, int(CHW[w])) * 8], I16)
    ein("dsti", [P, CH * 8], I16)
    ein("dstloc", [P, CH], BF16)
    ein("ohts", [P, NT * 5], F32)
    ein("ohcl", [P, NT * 2], F32)
    ein("vm", [P, NT * 2], F32)
    ein("wtab1", [D_IN, 80], BF16)
    ein("wsd2", [64, 16], BF16)
    ein("wbig", [P, 256], BF16)
    ein("tsw1", [64, 64], BF16)
    ein("clsw1", [64, 64], BF16)
    ein("tsw2", [64, 5], BF16)
    ein("clsw2", [64, 2], BF16)
    ein("b1r", [P, 64], F32)
    ein("b2r", [P, 64], F32)
    ein("tsb1c", [64, 1], F32)
    ein("clsb1c", [64, 1], F32)
    ein("tsb2c", [8, 1], F32)
    ein("clsb2c", [8, 1], F32)
    ein("iota", [P, P], BF16)
    identf_d = ein("identf", [P, P], F32)
    ein("ident", [P, P], BF16)
    ein("ones", [P, 1], F32)
    out_d = nc.dram_tensor("out", [1, 1], F32, kind="ExternalOutput")
    with tile.TileContext(nc) as tc:
        with tc.tile_pool(name="sp", bufs=1) as sp:
            t = sp.tile([1, 1], F32, tag="t")
            nc.sync.dma_start(out=t[:], in_=identf_d[0:1, 0:1])
            nc.sync.dma_start(out=out_d[:], in_=t[:])
    nc.compile()
    return nc

